# revision 20
# baseline (speedup 1.0000x reference)
"""Neural-HMM forward kernel for Trainium2 (8 NeuronCores, SPMD data-parallel over batch).

Math: the reference computes, per (b, t), a 64x64 transition matrix
A_t = 0.5*softmax(emb@W + b) + 0.5*softmax(unnorm_trans), then a log-space
scan h_t = logsumexp_i(h_{t-1}[i] + log A_t[i, j]).  Since every A_t is
row-stochastic, the scan is numerically safe in plain probability space:
p_t = A_t^T p_{t-1} with total mass conserved, h_t = log p_t.  That turns the
logsumexp-matmul chain into ordinary matmuls on the tensor engine.

Parallelization over the sequential scan: chunk T into blocks of 64 steps and
run *matrix* prefix chains N_s = A_s^T N_{s-1} (N_{-1}=I) per chunk -- all
chunks independent.  A cheap sequential pass over chunk-final products gives
each chunk's entry state q_c; every per-step state is then p = N_s q_c, a
bulk multiply-reduce on the vector engine.

v2 layout notes (vs the fp32r baseline):
 - main matmul runs in bf16 (1 cyc/row on the PE vs 3 for fp32-HIGH)
 - A is stored [(hh,i), (j, t2)] with t2 innermost so every exp ACTIVATE
   writes a contiguous 256-element run (the old (t2, j) layout scattered
   2B every 128B and cost ~1.4us per instruction)
 - the chain matmul lhsT reads j strided (512B) -- LDWEIGHTS streams one
   column per cycle regardless of stride
 - softmax denominator via strided-X tensor_reduce (1x mode is
   stride-insensitive), trans-add on gpsimd, per-step chain casts
   alternate vector/gpsimd
 - expansion multiply runs in-place on the chain buffer, reduction over
   entry states as a bf16 pairwise tree (2x mode) instead of 1x reduce
 - h is transposed on the PE before the output DMA so the HBM write is
   row-contiguous (the old transposed DMA cost ~3.4us per chunk)
"""

import math
import numpy as np
import ml_dtypes
import sys

sys.path.insert(0, "/opt/trn_rl_repo")

import concourse.bass as bass
import concourse.bacc as bacc
import concourse.tile as tile
from concourse import mybir
from concourse.bass_utils import run_bass_kernel_spmd

F32 = mybir.dt.float32
BF16 = mybir.dt.bfloat16

B, T, D, H = 16, 1024, 1024, 64
NCORES = 8
BLOC = B // NCORES          # batches per core = 2
NBLK = 4                    # time-blocks per core: (b, t-half)
BT = 512                    # timesteps per block
NCH = 8                     # chunks per block
L = 64                      # steps per chunk
NQ = 32                     # lhsT m-tiles (column-pair groups) in main matmul
NK = 8                      # contraction tiles (1024 / 128)


def build_bass():
    nc = bacc.Bacc(
        "TRN2", target_bir_lowering=False, debug=False, num_devices=NCORES
    )
    embT = nc.declare_dram_parameter("embT", [D, BLOC * T], BF16, isOutput=False)
    Wp = nc.declare_dram_parameter("Wp", [D, H * H], BF16, isOutput=False)
    bp = nc.declare_dram_parameter("bp", [128, 2 * NQ], F32, isOutput=False)
    trans_rep = nc.declare_dram_parameter("trans_rep", [128, H], BF16, isOutput=False)
    ident = nc.declare_dram_parameter("ident", [128, H], BF16, isOutput=False)
    identf = nc.declare_dram_parameter("identf", [H, H], F32, isOutput=False)
    onesr = nc.declare_dram_parameter("onesr", [1, 128], F32, isOutput=False)
    priors_col = nc.declare_dram_parameter("priors_col", [128, 1], F32, isOutput=False)
    priors_row = nc.declare_dram_parameter("priors_row", [1, H], F32, isOutput=False)
    out = nc.declare_dram_parameter("out", [BLOC * T, H], F32, isOutput=True)

    from contextlib import ExitStack

    with tile.TileContext(nc) as tc, ExitStack() as ctx:
        kernel_body(
            ctx, tc, embT, Wp, bp, trans_rep, ident, identf, onesr,
            priors_col, priors_row, out,
        )
    nc.finalize()
    return nc


def kernel_body(
    ctx, tc, embT, Wp, bp, trans_rep, ident, identf, onesr, priors_col,
    priors_row, out,
):
    nc = tc.nc

    const_pool = ctx.enter_context(tc.tile_pool(name="const", bufs=1))
    embt_pool = ctx.enter_context(tc.tile_pool(name="embt", bufs=2))
    w_pool = ctx.enter_context(tc.tile_pool(name="w", bufs=3))
    a_pool = ctx.enter_context(tc.tile_pool(name="a", bufs=2))
    n_pool = ctx.enter_context(tc.tile_pool(name="n", bufs=2))
    s_pool = ctx.enter_context(tc.tile_pool(name="s", bufs=2))
    stree_pool = ctx.enter_context(tc.tile_pool(name="stree", bufs=1))
    tree_pool = ctx.enter_context(tc.tile_pool(name="tree", bufs=2))
    p_pool = ctx.enter_context(tc.tile_pool(name="p", bufs=2))
    q_pool = ctx.enter_context(tc.tile_pool(name="q", bufs=2))
    qrep_pool = ctx.enter_context(tc.tile_pool(name="qrep", bufs=3))
    mm_psum = ctx.enter_context(tc.tile_pool(name="mmps", bufs=2, space="PSUM"))
    ch_psumA = ctx.enter_context(tc.tile_pool(name="chpsA", bufs=2, space="PSUM"))
    ch_psumB = ctx.enter_context(tc.tile_pool(name="chpsB", bufs=2, space="PSUM"))
    bnd_psum = ctx.enter_context(tc.tile_pool(name="bndps", bufs=1, space="PSUM"))
    tp_psum = ctx.enter_context(tc.tile_pool(name="tpps", bufs=1, space="PSUM"))

    # constants.  The small fp32 matmul operands (ones / identities / q) are
    # staged through a DVE copy so that every boundary fp32 matmul depends on
    # a single semaphore domain -- the fp32 self-loading LDWEIGHTS struct only
    # has one sync-wait slot.
    trans_sb = const_pool.tile([128, H], BF16)
    nc.sync.dma_start(trans_sb[:, :], trans_rep[:, :])
    transf_sb = const_pool.tile([128, H], F32)
    nc.vector.tensor_copy(transf_sb[:, :], trans_sb[:, :])
    ident_dma = const_pool.tile([128, H], BF16)
    nc.sync.dma_start(ident_dma[:, :], ident[:, :])
    ident_sb = const_pool.tile([128, H], BF16)
    nc.vector.tensor_copy(ident_sb[:, :], ident_dma[:, :])
    identf_dma = const_pool.tile([H, H], F32)
    nc.sync.dma_start(identf_dma[:, :], identf[:, :])
    identf_sb = const_pool.tile([H, H], F32)
    nc.vector.tensor_copy(identf_sb[:, :], identf_dma[:, :])
    ones_dma = const_pool.tile([1, 128], F32)
    nc.sync.dma_start(ones_dma[:, :], onesr[:, :])
    ones_sb = const_pool.tile([1, 128], F32)
    nc.vector.tensor_copy(ones_sb[:, :], ones_dma[:, :])
    pcol_sb = const_pool.tile([128, 1], F32)
    nc.sync.dma_start(pcol_sb[:, :], priors_col[:, :])
    prow_sb = const_pool.tile([1, H], F32)
    nc.sync.dma_start(prow_sb[:, :], priors_row[:, :])
    bcol_sb = const_pool.tile([128, 2 * NQ], F32)
    nc.sync.dma_start(bcol_sb[:, :], bp[:, :])

    # boundary state: q as a replicated column [128, 1] (both halves hold q)
    # and as a row [1, H].  Updated per chunk via tiny PE matmuls.  Kept in a
    # mutable box so the stage generators share the rolling value.
    qbox = {}
    qbox["col"] = q_pool.tile([128, 1], F32, tag="qcol", name="qcol")
    nc.vector.tensor_copy(qbox["col"][:, :], pcol_sb[:, :])
    qbox["row"] = q_pool.tile([1, H], F32, tag="qrow", name="qrow")
    nc.vector.tensor_copy(qbox["row"][:, :], prow_sb[:, :])

    def stage1_gen(blk):
        """Main matmul + exp for one block.  Yields every 2 contraction
        matmuls (~0.4us of PE work) so chain rounds of the previous block
        can be woven between them at a grain finer than their latency."""
        b = blk // 2
        th = blk % 2
        tcol0 = b * T + th * BT

        embt_sb = embt_pool.tile([128, NK * BT], BF16, tag="embt")
        # dest free dims (k, t); src embT[k*128 + p, tcol0 + t]
        nc.sync.dma_start(
            embt_sb[:, :].rearrange("p (k t) -> p k t", k=NK),
            embT[:, tcol0 : tcol0 + BT].rearrange("(k p) t -> p k t", p=128),
        )

        # A layout: [(hh, i) partitions, (j, t2) free] -- t2 innermost so the
        # exp writes are contiguous; chain lhsT reads j with stride 256.
        a_sb = a_pool.tile([128, H * 256], BF16, tag="a")
        av = a_sb[:, :].rearrange("p (j t) -> p j t", j=H)   # [128, 64, 256]
        avs[blk] = av
        yield

        for q in range(NQ):
            w_sb = w_pool.tile([128, NK * 128], BF16, tag="w")
            nc.sync.dma_start(
                w_sb[:, :].rearrange("p (k m) -> p k m", k=NK),
                Wp[:, q * 128 : (q + 1) * 128].rearrange("(k p) m -> p k m", p=128),
            )
            ps = mm_psum.tile([128, BT], F32, tag="mm")
            for k in range(NK):
                nc.tensor.matmul(
                    ps[:, :],
                    w_sb[:, k * 128 : (k + 1) * 128],
                    embt_sb[:, k * BT : (k + 1) * BT],
                    start=(k == 0),
                    stop=(k == NK - 1),
                )
                if k % 2 == 1:
                    yield
            # exp(psum + bias) -> E at av[(hh, i), j = q + 32h, :]
            for h in range(2):
                j = q + 32 * h
                for hh in range(2):
                    nc.scalar.activation(
                        av[hh * 64 : hh * 64 + 64, j : j + 1, :],
                        ps[h * 64 : h * 64 + 64, hh * 256 : hh * 256 + 256].rearrange(
                            "p (u t) -> p u t", u=1
                        ),
                        mybir.ActivationFunctionType.Exp,
                        bias=bcol_sb[h * 64 : h * 64 + 64, 2 * q + h : 2 * q + h + 1],
                    )
            yield

    def stage234_gen(blk):
        """Softmax assembly, chain, boundary scan, expansion and output for
        one block.  The assembly and the chain are split into t-halves /
        chunk-groups X (f 0,1) and Y (f 2,3) so Y's work overlaps X's round
        latency; yields are one chain round (or one assembly piece) each."""
        b = blk // 2
        th = blk % 2
        av = avs.pop(blk)
        a_flat = av  # [128, 64, 256]

        if blk == 2:
            # reset boundary state to priors for the new batch element
            qbox["col"] = q_pool.tile([128, 1], F32, tag="qcol", name="qcol")
            nc.vector.tensor_copy(qbox["col"][:, :], pcol_sb[:, :])
            qbox["row"] = q_pool.tile([1, H], F32, tag="qrow", name="qrow")
            nc.vector.tensor_copy(qbox["row"][:, :], prow_sb[:, :])

        # ---- softmax assembly, one t-half (128 columns) at a time:
        # S = sum_j E (pairwise bf16 tree), r = 0.5/S, A = E*r + 0.5*trans
        s_sb = s_pool.tile([128, 256], F32, tag="s")
        r_sb = s_pool.tile([128, 256], BF16, tag="r")
        tr = stree_pool.tile([128, 32 * 256], BF16, tag="stree")
        tv = tr[:, :].rearrange("p (j t) -> p j t", j=32)

        def asm_half(t0):
            ts = slice(t0, t0 + 128)
            nc.vector.tensor_tensor(
                tv[:, :, ts], av[:, 0:32, ts], av[:, 32:64, ts],
                op=mybir.AluOpType.add,
            )
            yield
            w = 16
            while w >= 2:
                nc.vector.tensor_tensor(
                    tv[:, 0:w, ts], tv[:, 0:w, ts], tv[:, w : 2 * w, ts],
                    op=mybir.AluOpType.add,
                )
                w //= 2
            yield
            nc.vector.tensor_tensor(
                s_sb[:, ts].rearrange("p (u t) -> p u t", u=1),
                tv[:, 0:1, ts], tv[:, 1:2, ts], op=mybir.AluOpType.add,
            )
            nc.vector.reciprocal(s_sb[:, ts], s_sb[:, ts])
            nc.vector.tensor_scalar_mul(r_sb[:, ts], s_sb[:, ts], 0.5)
            yield
            nc.vector.tensor_tensor(
                av[:, :, ts],
                av[:, :, ts],
                r_sb[:, ts].rearrange("p (u t) -> p u t", u=1).broadcast_to([128, H, 128]),
                op=mybir.AluOpType.mult,
            )
            yield
            nc.vector.tensor_tensor(
                av[:, :, ts],
                av[:, :, ts],
                trans_sb[:, :].rearrange("p (j u) -> p j u", u=1).broadcast_to([128, H, 128]),
                op=mybir.AluOpType.add,
            )
            yield

        # ---- chain rounds.  group g covers chunks f = 2g, 2g+1 (both hh);
        # X casts on the vector engine, Y casts on the scalar engine so the
        # two groups' round latencies overlap.
        n_sb = n_pool.tile([128, 4 * L * H], BF16, tag="n")
        nv = n_sb[:, :].rearrange("p (f s n) -> p f s n", f=4, s=L)
        first_chunk_global = th == 0

        def chain_round(g, s):
            pool = ch_psumA if g == 0 else ch_psumB
            cp = pool.tile([128, 128], F32, tag=f"chp{g}")
            for hh in range(2):
                for fi in range(2):
                    f = 2 * g + fi
                    c = hh * 4 + f
                    if s == 0:
                        if first_chunk_global and c == 0:
                            continue  # slot 0 of the very first chunk = identity
                        rhs = ident_sb[hh * 64 : hh * 64 + 64, :]
                    else:
                        rhs = nv[hh * 64 : hh * 64 + 64, f, s - 1, :]
                    nc.tensor.matmul(
                        cp[hh * 64 : hh * 64 + 64, fi * 64 : fi * 64 + 64],
                        av[hh * 64 : hh * 64 + 64, :, f * 64 + s],
                        rhs,
                        start=True,
                        stop=True,
                        tile_position=(hh * 64, hh * 64),
                    )
            cpv = cp[:, :].rearrange("p (fi n) -> p fi n", fi=2)
            dst = nv[:, 2 * g : 2 * g + 2, s, :]
            if g == 0 and s == 0 and first_chunk_global:
                nc.vector.tensor_copy(nv[0:64, 0, 0, :], ident_sb[0:64, :])
                nc.vector.tensor_copy(nv[0:64, 1, 0, :], cpv[0:64, 1, :])
                nc.vector.tensor_copy(nv[64:128, 0:2, 0, :], cpv[64:128, :, :])
            elif g == 0:
                nc.vector.tensor_copy(dst, cpv[:, :, :])
            else:
                nc.scalar.copy(dst, cpv[:, :, :])

        # half-0 assembly, then X rounds 0..7, then half-1 assembly, then
        # alternate X/Y rounds
        yield from asm_half(0)
        for s in range(8):
            chain_round(0, s)
            yield
        yield from asm_half(128)
        for s in range(8):
            chain_round(1, s)
            yield
        for s in range(8, L):
            chain_round(0, s)
            yield
            chain_round(1, s)
            yield

        # ---- stage 3: boundary scan.  The chunk products M_63 are all
        # independent -- compute them first so the serial part is only the
        # eight tiny qn matmul+copy pairs; q_row/q_rep replication hangs off
        # each q_col and is emitted after the serial core.
        qreps = []
        for f in range(4):
            qr = qrep_pool.tile([128, H], BF16, tag=f"qr{f}")
            qreps.append(qr)
        m63s = []
        for c in range(NCH):
            hh, f = c // 4, c % 4
            bnd = bnd_psum.tile([128, 256], F32, tag="bnd")
            nc.tensor.matmul(
                bnd[hh * 64 : hh * 64 + 64, H : 2 * H],
                nv[hh * 64 : hh * 64 + 64, f, L - 2, :],
                av[hh * 64 : hh * 64 + 64, :, f * 64 + (L - 1)],
                start=True,
                stop=True,
                tile_position=(hh * 64, hh * 64),
            )
            m63_sb = q_pool.tile([128, H], F32, tag=f"m63sb{c}")
            nc.vector.tensor_copy(
                m63_sb[hh * 64 : hh * 64 + 64, :],
                bnd[hh * 64 : hh * 64 + 64, H : 2 * H],
            )
            m63s.append(m63_sb)
            yield
        qcols = [qbox["col"]]
        qrows = [qbox["row"]]
        for c in range(NCH):
            hh = c // 4
            bnd = bnd_psum.tile([128, 256], F32, tag="bnd")
            nc.tensor.matmul(
                bnd[0:H, 128 : 129],
                m63s[c][hh * 64 : hh * 64 + 64, :],
                qcols[c][hh * 64 : hh * 64 + 64, :],
                start=True,
                stop=True,
                tile_position=(hh * 64, 0),
            )
            qc = q_pool.tile([128, 1], F32, tag=f"qcol{c}", name="qc")
            nc.vector.tensor_copy(qc[0:64, :], bnd[0:H, 128:129])
            nc.vector.tensor_copy(qc[64:128, :], bnd[0:H, 128:129])
            qcols.append(qc)
            yield
        qbox["col"] = qcols[NCH]
        # off the serial path: q_row(c) = q_col(c).T @ I, then
        # q_rep(c) = ones.T @ q_row(c) into the expansion operand
        for c in range(NCH):
            hh, f = c // 4, c % 4
            bnd = bnd_psum.tile([128, 256], F32, tag="bnd")
            if c > 0:
                nc.tensor.matmul(
                    bnd[0:1, 192:256], qcols[c][0:64, :], identf_sb[:, :],
                    start=True, stop=True,
                )
                qr_row = q_pool.tile([1, H], F32, tag="qrow", name="qrow")
                nc.vector.tensor_copy(qr_row[:, :], bnd[0:1, 192:256])
            else:
                qr_row = qrows[0]
            nc.tensor.matmul(
                bnd[:, 0:H], ones_sb[:, :], qr_row[:, :], start=True, stop=True
            )
            nc.vector.tensor_copy(
                qreps[f][hh * 64 : hh * 64 + 64, :],
                bnd[hh * 64 : hh * 64 + 64, 0:H],
            )
            if c == NCH - 1:
                qbox["row"] = qr_row
            yield "slow"

        # ---- stage 4: expansion p = N_s q_c, then batched Ln, PE transpose
        # and row-contiguous output DMA.  gpsimd is ~2.3x slower per element,
        # so it only gets the latency-tolerant trees of two chunks -- and
        # none at all in the last block (they would gate the final DMAs).
        p_fs = []
        for f in range(4):
            nc.vector.tensor_tensor(
                nv[:, f, :, :],
                nv[:, f, :, :],
                qreps[f][:, :].rearrange("p (u n) -> p u n", u=1).broadcast_to([128, L, H]),
                op=mybir.AluOpType.mult,
            )
            treng = nc.vector if (f < 2 or blk == NBLK - 1) else nc.gpsimd
            tr2 = tree_pool.tile([128, L * 32], BF16, tag="tree")
            tv2 = tr2[:, :].rearrange("p (s n) -> p s n", s=L)
            treng.tensor_tensor(
                tv2[:, :, :], nv[:, f, :, 0:32], nv[:, f, :, 32:64],
                op=mybir.AluOpType.add,
            )
            w = 16
            while w >= 2:
                treng.tensor_tensor(
                    tv2[:, :, 0:w], tv2[:, :, 0:w], tv2[:, :, w : 2 * w],
                    op=mybir.AluOpType.add,
                )
                w //= 2
            p_f = p_pool.tile([128, L], F32, tag=f"pf{f}")
            treng.tensor_tensor(
                p_f[:, :].rearrange("p (s u) -> p s u", u=1),
                tv2[:, :, 0:1], tv2[:, :, 1:2],
                op=mybir.AluOpType.add,
            )
            p_fs.append(p_f)
            yield "slow"

        # batched Ln so the scalar engine switches activation tables only
        # twice per block (Exp set <-> Ln set)
        h_fs = {}
        for f in range(4):
            for hh in range(2):
                h_f = p_pool.tile([64, L], F32, tag=f"hf{f}{hh}")
                nc.scalar.activation(
                    h_f[:, :], p_fs[f][hh * 64 : hh * 64 + 64, :],
                    mybir.ActivationFunctionType.Ln,
                )
                h_fs[(f, hh)] = h_f
        yield
        for f in range(4):
            for hh in range(2):
                c = hh * 4 + f
                r0 = b * T + th * BT + c * L
                tp = tp_psum.tile([128, H], F32, tag="tp")
                nc.tensor.transpose(tp[0:64, :], h_fs[(f, hh)][:, :], identf_sb[:, :])
                ht = p_pool.tile([128, H], F32, tag="ht")
                nc.scalar.copy(ht[0:64, :], tp[0:64, :])
                nc.sync.dma_start(out[r0 : r0 + L, :], ht[0:64, :])
            yield

    # ---- weave: stage1(blk) emission is interleaved with stage234(blk-1)
    # so the PE's in-order queue alternates big main-matmul bursts with the
    # tiny latency-bound chain matmuls (keeps the PE dense and HAM-warm).
    avs = {}
    DONE = object()

    def adv(g, n=1):
        for _ in range(n):
            if next(g, DONE) is DONE:
                return False
        return True

    prev = None
    for blk in range(NBLK):
        cur = stage1_gen(blk)
        adv(cur)                      # embT DMA + a_sb alloc
        alive = prev is not None
        debt = 0
        while True:
            if not adv(cur):
                break
            if debt > 0:
                debt -= 1
                continue
            if alive:
                tok = next(prev, DONE)
                if tok is DONE:
                    alive = False
                elif tok == "slow":
                    debt = 2      # give slow units ~3 pieces of PE cover
        while alive:
            alive = adv(prev)
        prev = stage234_gen(blk)
    while adv(prev):
        pass


def kernel(emb, W, b, unnorm_trans, state_priors):
    emb = np.asarray(emb, dtype=np.float32)
    W = np.asarray(W, dtype=np.float32)
    b = np.asarray(b, dtype=np.float32)
    unnorm_trans = np.asarray(unnorm_trans, dtype=np.float32)
    state_priors = np.asarray(state_priors, dtype=np.float32)

    # host-side constants
    ut = unnorm_trans - unnorm_trans.max(axis=-1, keepdims=True)
    e = np.exp(ut)
    trans_half = (0.5 * e / e.sum(axis=-1, keepdims=True)).astype(np.float32)
    trans_rep = np.tile(trans_half, (2, 1)).astype(ml_dtypes.bfloat16)
    ident = np.tile(np.eye(H, dtype=np.float32), (2, 1)).astype(ml_dtypes.bfloat16)
    identf = np.eye(H, dtype=np.float32)
    onesr = np.ones((1, 128), dtype=np.float32)
    pr = np.exp(state_priors).astype(np.float32)
    priors_col = np.tile(pr[:, None], (2, 1)).astype(np.float32)  # [128, 1]
    priors_row = pr[None, :].astype(np.float32)

    # permute W columns so each m-tile q covers cols {(i, q), (i, q+32)} contiguously
    Wp = (
        W.reshape(D, H, H)
        .transpose(0, 2, 1)          # [d, j, i]
        .reshape(D, 2, 32, H)        # j = jsel*32 + q
        .transpose(0, 2, 1, 3)       # [d, q, jsel, i]
        .reshape(D, H * H)
    )
    Wp = np.ascontiguousarray(Wp).astype(ml_dtypes.bfloat16)
    # bias columns: bp[h*64 + i, 2q + h] = b[i*64 + q + 32h]  (tiled)
    bp64 = np.empty((H, 2 * 32), dtype=np.float32)
    br = b.reshape(H, H)
    for q in range(32):
        for h in range(2):
            bp64[:, 2 * q + h] = br[:, q + 32 * h]
    bp = np.ascontiguousarray(np.tile(bp64, (2, 1)))

    nc = build_bass()

    in_maps = []
    for core in range(NCORES):
        emb_c = emb[core * BLOC : (core + 1) * BLOC]          # [2, T, D]
        embT_c = np.ascontiguousarray(
            emb_c.transpose(2, 0, 1).reshape(D, BLOC * T)
        ).astype(ml_dtypes.bfloat16)
        in_maps.append(
            {
                "embT": embT_c,
                "Wp": Wp,
                "bp": bp,
                "trans_rep": trans_rep,
                "ident": ident,
                "identf": identf,
                "onesr": onesr,
                "priors_col": priors_col,
                "priors_row": priors_row,
            }
        )

    import os

    trace = bool(int(os.environ.get("KERNEL_TRACE", "0")))
    res = run_bass_kernel_spmd(nc, in_maps, list(range(NCORES)), trace=trace)
    global LAST_RESULTS
    LAST_RESULTS = res
    if trace and res.exec_time_ns is not None:
        print(f"HW exec time: {res.exec_time_ns} ns")
        print(f"  mean across cores: {res.mean_exec_time_ns} ns")
    outs = [res.results[i]["out"].reshape(BLOC, T, H) for i in range(NCORES)]
    h = np.concatenate(outs, axis=0).astype(np.float32)
    h[:, 0, :] = state_priors[None, :]
    return h


# revision 21
# speedup vs baseline: 1.0163x; 1.0163x over previous
"""Neural-HMM forward kernel for Trainium2 (8 NeuronCores, SPMD data-parallel over batch).

Math: the reference computes, per (b, t), a 64x64 transition matrix
A_t = 0.5*softmax(emb@W + b) + 0.5*softmax(unnorm_trans), then a log-space
scan h_t = logsumexp_i(h_{t-1}[i] + log A_t[i, j]).  Since every A_t is
row-stochastic, the scan is numerically safe in plain probability space:
p_t = A_t^T p_{t-1} with total mass conserved, h_t = log p_t.  That turns the
logsumexp-matmul chain into ordinary matmuls on the tensor engine.

Parallelization over the sequential scan: chunk T into blocks of 64 steps and
run *matrix* prefix chains N_s = A_s^T N_{s-1} (N_{-1}=I) per chunk -- all
chunks independent.  A cheap sequential pass over chunk-final products gives
each chunk's entry state q_c; every per-step state is then p = N_s q_c, a
bulk multiply-reduce on the vector engine.

v2 layout notes (vs the fp32r baseline):
 - main matmul runs in bf16 (1 cyc/row on the PE vs 3 for fp32-HIGH)
 - A is stored [(hh,i), (j, t2)] with t2 innermost so every exp ACTIVATE
   writes a contiguous 256-element run (the old (t2, j) layout scattered
   2B every 128B and cost ~1.4us per instruction)
 - the chain matmul lhsT reads j strided (512B) -- LDWEIGHTS streams one
   column per cycle regardless of stride
 - softmax denominator via strided-X tensor_reduce (1x mode is
   stride-insensitive), trans-add on gpsimd, per-step chain casts
   alternate vector/gpsimd
 - expansion multiply runs in-place on the chain buffer, reduction over
   entry states as a bf16 pairwise tree (2x mode) instead of 1x reduce
 - h is transposed on the PE before the output DMA so the HBM write is
   row-contiguous (the old transposed DMA cost ~3.4us per chunk)
"""

import math
import numpy as np
import ml_dtypes
import sys

sys.path.insert(0, "/opt/trn_rl_repo")

import concourse.bass as bass
import concourse.bacc as bacc
import concourse.tile as tile
from concourse import mybir
from concourse.bass_utils import run_bass_kernel_spmd

F32 = mybir.dt.float32
BF16 = mybir.dt.bfloat16

B, T, D, H = 16, 1024, 1024, 64
NCORES = 8
BLOC = B // NCORES          # batches per core = 2
NBLK = 4                    # time-blocks per core: (b, t-half)
BT = 512                    # timesteps per block
NCH = 8                     # chunks per block
L = 64                      # steps per chunk
NQ = 32                     # lhsT m-tiles (column-pair groups) in main matmul
NK = 8                      # contraction tiles (1024 / 128)


def build_bass():
    nc = bacc.Bacc(
        "TRN2", target_bir_lowering=False, debug=False, num_devices=NCORES
    )
    embT = nc.declare_dram_parameter("embT", [D, BLOC * T], BF16, isOutput=False)
    Wp = nc.declare_dram_parameter("Wp", [D, H * H], BF16, isOutput=False)
    bp = nc.declare_dram_parameter("bp", [128, 2 * NQ], F32, isOutput=False)
    trans_rep = nc.declare_dram_parameter("trans_rep", [128, H], BF16, isOutput=False)
    ident = nc.declare_dram_parameter("ident", [128, H], BF16, isOutput=False)
    identf = nc.declare_dram_parameter("identf", [H, H], F32, isOutput=False)
    onesr = nc.declare_dram_parameter("onesr", [1, 128], F32, isOutput=False)
    priors_col = nc.declare_dram_parameter("priors_col", [128, 1], F32, isOutput=False)
    priors_row = nc.declare_dram_parameter("priors_row", [1, H], F32, isOutput=False)
    out = nc.declare_dram_parameter("out", [BLOC * T, H], F32, isOutput=True)

    from contextlib import ExitStack

    with tile.TileContext(nc) as tc, ExitStack() as ctx:
        kernel_body(
            ctx, tc, embT, Wp, bp, trans_rep, ident, identf, onesr,
            priors_col, priors_row, out,
        )
    nc.finalize()
    return nc


def kernel_body(
    ctx, tc, embT, Wp, bp, trans_rep, ident, identf, onesr, priors_col,
    priors_row, out,
):
    nc = tc.nc

    const_pool = ctx.enter_context(tc.tile_pool(name="const", bufs=1))
    embt_pool = ctx.enter_context(tc.tile_pool(name="embt", bufs=2))
    w_pool = ctx.enter_context(tc.tile_pool(name="w", bufs=3))
    a_pool = ctx.enter_context(tc.tile_pool(name="a", bufs=2))
    n_pool = ctx.enter_context(tc.tile_pool(name="n", bufs=2))
    s_pool = ctx.enter_context(tc.tile_pool(name="s", bufs=2))
    stree_pool = ctx.enter_context(tc.tile_pool(name="stree", bufs=1))
    tree_pool = ctx.enter_context(tc.tile_pool(name="tree", bufs=2))
    p_pool = ctx.enter_context(tc.tile_pool(name="p", bufs=2))
    q_pool = ctx.enter_context(tc.tile_pool(name="q", bufs=2))
    qrep_pool = ctx.enter_context(tc.tile_pool(name="qrep", bufs=3))
    mm_psum = ctx.enter_context(tc.tile_pool(name="mmps", bufs=2, space="PSUM"))
    ch_psumA = ctx.enter_context(tc.tile_pool(name="chpsA", bufs=2, space="PSUM"))
    ch_psumB = ctx.enter_context(tc.tile_pool(name="chpsB", bufs=2, space="PSUM"))
    bnd_psum = ctx.enter_context(tc.tile_pool(name="bndps", bufs=1, space="PSUM"))
    tp_psum = ctx.enter_context(tc.tile_pool(name="tpps", bufs=1, space="PSUM"))

    # constants.  The small fp32 matmul operands (ones / identities / q) are
    # staged through a DVE copy so that every boundary fp32 matmul depends on
    # a single semaphore domain -- the fp32 self-loading LDWEIGHTS struct only
    # has one sync-wait slot.
    trans_sb = const_pool.tile([128, H], BF16)
    nc.sync.dma_start(trans_sb[:, :], trans_rep[:, :])
    transf_sb = const_pool.tile([128, H], F32)
    nc.vector.tensor_copy(transf_sb[:, :], trans_sb[:, :])
    ident_dma = const_pool.tile([128, H], BF16)
    nc.sync.dma_start(ident_dma[:, :], ident[:, :])
    ident_sb = const_pool.tile([128, H], BF16)
    nc.vector.tensor_copy(ident_sb[:, :], ident_dma[:, :])
    identf_dma = const_pool.tile([H, H], F32)
    nc.sync.dma_start(identf_dma[:, :], identf[:, :])
    identf_sb = const_pool.tile([H, H], F32)
    nc.vector.tensor_copy(identf_sb[:, :], identf_dma[:, :])
    ones_dma = const_pool.tile([1, 128], F32)
    nc.sync.dma_start(ones_dma[:, :], onesr[:, :])
    ones_sb = const_pool.tile([1, 128], F32)
    nc.vector.tensor_copy(ones_sb[:, :], ones_dma[:, :])
    pcol_sb = const_pool.tile([128, 1], F32)
    nc.sync.dma_start(pcol_sb[:, :], priors_col[:, :])
    prow_sb = const_pool.tile([1, H], F32)
    nc.sync.dma_start(prow_sb[:, :], priors_row[:, :])
    bcol_sb = const_pool.tile([128, 2 * NQ], F32)
    nc.sync.dma_start(bcol_sb[:, :], bp[:, :])

    # boundary state: q as a replicated column [128, 1] (both halves hold q)
    # and as a row [1, H].  Updated per chunk via tiny PE matmuls.  Kept in a
    # mutable box so the stage generators share the rolling value.
    qbox = {}
    qbox["col"] = q_pool.tile([128, 1], F32, tag="qcol", name="qcol")
    nc.vector.tensor_copy(qbox["col"][:, :], pcol_sb[:, :])
    qbox["row"] = q_pool.tile([1, H], F32, tag="qrow", name="qrow")
    nc.vector.tensor_copy(qbox["row"][:, :], prow_sb[:, :])

    def stage1_gen(blk):
        """Main matmul + exp for one block.  Yields every 2 contraction
        matmuls (~0.4us of PE work) so chain rounds of the previous block
        can be woven between them at a grain finer than their latency."""
        b = blk // 2
        th = blk % 2
        tcol0 = b * T + th * BT

        embt_sb = embt_pool.tile([128, NK * BT], BF16, tag="embt")
        # dest free dims (k, t); src embT[k*128 + p, tcol0 + t]
        nc.sync.dma_start(
            embt_sb[:, :].rearrange("p (k t) -> p k t", k=NK),
            embT[:, tcol0 : tcol0 + BT].rearrange("(k p) t -> p k t", p=128),
        )

        # A layout: [(hh, i) partitions, (j, t2) free] -- t2 innermost so the
        # exp writes are contiguous; chain lhsT reads j with stride 256.
        a_sb = a_pool.tile([128, H * 256], BF16, tag="a")
        av = a_sb[:, :].rearrange("p (j t) -> p j t", j=H)   # [128, 64, 256]
        avs[blk] = av
        yield

        for q in range(NQ):
            w_sb = w_pool.tile([128, NK * 128], BF16, tag="w")
            nc.sync.dma_start(
                w_sb[:, :].rearrange("p (k m) -> p k m", k=NK),
                Wp[:, q * 128 : (q + 1) * 128].rearrange("(k p) m -> p k m", p=128),
            )
            ps = mm_psum.tile([128, BT], F32, tag="mm")
            for k in range(NK):
                nc.tensor.matmul(
                    ps[:, :],
                    w_sb[:, k * 128 : (k + 1) * 128],
                    embt_sb[:, k * BT : (k + 1) * BT],
                    start=(k == 0),
                    stop=(k == NK - 1),
                )
                if k % 2 == 1:
                    yield
            # exp(psum + bias) -> E at av[(hh, i), j = q + 32h, :]
            for h in range(2):
                j = q + 32 * h
                for hh in range(2):
                    nc.scalar.activation(
                        av[hh * 64 : hh * 64 + 64, j : j + 1, :],
                        ps[h * 64 : h * 64 + 64, hh * 256 : hh * 256 + 256].rearrange(
                            "p (u t) -> p u t", u=1
                        ),
                        mybir.ActivationFunctionType.Exp,
                        bias=bcol_sb[h * 64 : h * 64 + 64, 2 * q + h : 2 * q + h + 1],
                    )
            yield

    def stage234_gen(blk):
        """Softmax assembly, chain, boundary scan, expansion and output for
        one block.  The assembly and the chain are split into t-halves /
        chunk-groups X (f 0,1) and Y (f 2,3) so Y's work overlaps X's round
        latency; yields are one chain round (or one assembly piece) each."""
        b = blk // 2
        th = blk % 2
        av = avs.pop(blk)
        a_flat = av  # [128, 64, 256]

        if blk == 2:
            # reset boundary state to priors for the new batch element
            qbox["col"] = q_pool.tile([128, 1], F32, tag="qcol", name="qcol")
            nc.vector.tensor_copy(qbox["col"][:, :], pcol_sb[:, :])
            qbox["row"] = q_pool.tile([1, H], F32, tag="qrow", name="qrow")
            nc.vector.tensor_copy(qbox["row"][:, :], prow_sb[:, :])

        # ---- softmax assembly, one t-half (128 columns) at a time:
        # S = sum_j E (pairwise bf16 tree), r = 0.5/S, A = E*r + 0.5*trans
        s_sb = s_pool.tile([128, 256], F32, tag="s")
        r_sb = s_pool.tile([128, 256], BF16, tag="r")
        tr = stree_pool.tile([128, 32 * 256], BF16, tag="stree")
        tv = tr[:, :].rearrange("p (j t) -> p j t", j=32)

        def asm_half(t0):
            ts = slice(t0, t0 + 128)
            nc.vector.tensor_tensor(
                tv[:, :, ts], av[:, 0:32, ts], av[:, 32:64, ts],
                op=mybir.AluOpType.add,
            )
            yield
            w = 16
            while w >= 2:
                nc.vector.tensor_tensor(
                    tv[:, 0:w, ts], tv[:, 0:w, ts], tv[:, w : 2 * w, ts],
                    op=mybir.AluOpType.add,
                )
                w //= 2
            yield
            nc.vector.tensor_tensor(
                s_sb[:, ts].rearrange("p (u t) -> p u t", u=1),
                tv[:, 0:1, ts], tv[:, 1:2, ts], op=mybir.AluOpType.add,
            )
            nc.vector.reciprocal(s_sb[:, ts], s_sb[:, ts])
            nc.vector.tensor_scalar_mul(r_sb[:, ts], s_sb[:, ts], 0.5)
            yield
            nc.vector.tensor_tensor(
                av[:, :, ts],
                av[:, :, ts],
                r_sb[:, ts].rearrange("p (u t) -> p u t", u=1).broadcast_to([128, H, 128]),
                op=mybir.AluOpType.mult,
            )
            yield
            nc.vector.tensor_tensor(
                av[:, :, ts],
                av[:, :, ts],
                trans_sb[:, :].rearrange("p (j u) -> p j u", u=1).broadcast_to([128, H, 128]),
                op=mybir.AluOpType.add,
            )
            yield

        # ---- chain rounds.  group g covers chunks f = 2g, 2g+1 (both hh);
        # X casts on the vector engine, Y casts on the scalar engine so the
        # two groups' round latencies overlap.
        n_sb = n_pool.tile([128, 4 * L * H], BF16, tag="n")
        nv = n_sb[:, :].rearrange("p (f s n) -> p f s n", f=4, s=L)
        first_chunk_global = th == 0

        def chain_round(g, s):
            pool = ch_psumA if g == 0 else ch_psumB
            cp = pool.tile([128, 128], F32, tag=f"chp{g}")
            for hh in range(2):
                for fi in range(2):
                    f = 2 * g + fi
                    c = hh * 4 + f
                    if s == 0:
                        if first_chunk_global and c == 0:
                            continue  # slot 0 of the very first chunk = identity
                        rhs = ident_sb[hh * 64 : hh * 64 + 64, :]
                    else:
                        rhs = nv[hh * 64 : hh * 64 + 64, f, s - 1, :]
                    nc.tensor.matmul(
                        cp[hh * 64 : hh * 64 + 64, fi * 64 : fi * 64 + 64],
                        av[hh * 64 : hh * 64 + 64, :, f * 64 + s],
                        rhs,
                        start=True,
                        stop=True,
                        tile_position=(hh * 64, hh * 64),
                    )
            cpv = cp[:, :].rearrange("p (fi n) -> p fi n", fi=2)
            dst = nv[:, 2 * g : 2 * g + 2, s, :]
            if g == 0 and s == 0 and first_chunk_global:
                nc.vector.tensor_copy(nv[0:64, 0, 0, :], ident_sb[0:64, :])
                nc.vector.tensor_copy(nv[0:64, 1, 0, :], cpv[0:64, 1, :])
                nc.vector.tensor_copy(nv[64:128, 0:2, 0, :], cpv[64:128, :, :])
            elif g == 0:
                nc.vector.tensor_copy(dst, cpv[:, :, :])
            else:
                nc.scalar.copy(dst, cpv[:, :, :])

        # half-0 assembly, then X rounds 0..7, then half-1 assembly, then
        # alternate X/Y rounds
        yield from asm_half(0)
        for s in range(8):
            chain_round(0, s)
            yield
        yield from asm_half(128)
        for s in range(8):
            chain_round(1, s)
            yield
        for s in range(8, L):
            chain_round(0, s)
            yield
            chain_round(1, s)
            yield

        # ---- stage 3: boundary scan + q replicas
        qreps = []
        for f in range(4):
            qr = qrep_pool.tile([128, H], BF16, tag=f"qr{f}")
            qreps.append(qr)
        for c in range(NCH):
            hh, f = c // 4, c % 4
            # replicate current q into the expansion operand rows for chunk c:
            # q_rep = ones.T @ q_row (PE), then copy psum half into qreps[f]
            bnd = bnd_psum.tile([128, 256], F32, tag="bnd")
            nc.tensor.matmul(
                bnd[:, 0:H], ones_sb[:, :], qbox["row"][:, :], start=True, stop=True
            )
            nc.vector.tensor_copy(
                qreps[f][hh * 64 : hh * 64 + 64, :],
                bnd[hh * 64 : hh * 64 + 64, 0:H],
            )
            # chunk-c full product M_63 = N_62.T @ A_63 (one extra chain-style MM)
            nc.tensor.matmul(
                bnd[hh * 64 : hh * 64 + 64, H : 2 * H],
                nv[hh * 64 : hh * 64 + 64, f, L - 2, :],
                av[hh * 64 : hh * 64 + 64, :, f * 64 + (L - 1)],
                start=True,
                stop=True,
                tile_position=(hh * 64, hh * 64),
            )
            m63_sb = q_pool.tile([128, H], F32, tag="m63sb")
            nc.vector.tensor_copy(
                m63_sb[hh * 64 : hh * 64 + 64, :],
                bnd[hh * 64 : hh * 64 + 64, H : 2 * H],
            )
            # boundary advance: qn_col = M_63.T @ q  ([64, 1], psum base 0)
            nc.tensor.matmul(
                bnd[0:H, 128 : 129],
                m63_sb[hh * 64 : hh * 64 + 64, :],
                qbox["col"][hh * 64 : hh * 64 + 64, :],
                start=True,
                stop=True,
                tile_position=(hh * 64, 0),
            )
            qbox["col"] = q_pool.tile([128, 1], F32, tag="qcol", name="qcol")
            nc.vector.tensor_copy(qbox["col"][0:64, :], bnd[0:H, 128:129])
            nc.vector.tensor_copy(qbox["col"][64:128, :], bnd[0:H, 128:129])
            # row form: q_row = q_col.T @ I
            nc.tensor.matmul(
                bnd[0:1, 192:256], qbox["col"][0:64, :], identf_sb[:, :],
                start=True, stop=True,
            )
            qbox["row"] = q_pool.tile([1, H], F32, tag="qrow", name="qrow")
            nc.vector.tensor_copy(qbox["row"][:, :], bnd[0:1, 192:256])
            yield "slow"

        # ---- stage 4: expansion p = N_s q_c, then batched Ln, PE transpose
        # and row-contiguous output DMA.  gpsimd is ~2.3x slower per element,
        # so it only gets the latency-tolerant trees of two chunks -- and
        # none at all in the last block (they would gate the final DMAs).
        p_fs = []
        for f in range(4):
            nc.vector.tensor_tensor(
                nv[:, f, :, :],
                nv[:, f, :, :],
                qreps[f][:, :].rearrange("p (u n) -> p u n", u=1).broadcast_to([128, L, H]),
                op=mybir.AluOpType.mult,
            )
            treng = nc.vector if (f < 2 or blk == NBLK - 1) else nc.gpsimd
            tr2 = tree_pool.tile([128, L * 32], BF16, tag="tree")
            tv2 = tr2[:, :].rearrange("p (s n) -> p s n", s=L)
            treng.tensor_tensor(
                tv2[:, :, :], nv[:, f, :, 0:32], nv[:, f, :, 32:64],
                op=mybir.AluOpType.add,
            )
            w = 16
            while w >= 2:
                treng.tensor_tensor(
                    tv2[:, :, 0:w], tv2[:, :, 0:w], tv2[:, :, w : 2 * w],
                    op=mybir.AluOpType.add,
                )
                w //= 2
            p_f = p_pool.tile([128, L], F32, tag=f"pf{f}")
            treng.tensor_tensor(
                p_f[:, :].rearrange("p (s u) -> p s u", u=1),
                tv2[:, :, 0:1], tv2[:, :, 1:2],
                op=mybir.AluOpType.add,
            )
            p_fs.append(p_f)
            yield "slow"

        # batched Ln so the scalar engine switches activation tables only
        # twice per block (Exp set <-> Ln set)
        h_fs = {}
        for f in range(4):
            for hh in range(2):
                h_f = p_pool.tile([64, L], F32, tag=f"hf{f}{hh}")
                nc.scalar.activation(
                    h_f[:, :], p_fs[f][hh * 64 : hh * 64 + 64, :],
                    mybir.ActivationFunctionType.Ln,
                )
                h_fs[(f, hh)] = h_f
        yield
        for f in range(4):
            for hh in range(2):
                c = hh * 4 + f
                r0 = b * T + th * BT + c * L
                tp = tp_psum.tile([128, H], F32, tag="tp")
                nc.tensor.transpose(tp[0:64, :], h_fs[(f, hh)][:, :], identf_sb[:, :])
                ht = p_pool.tile([128, H], F32, tag="ht")
                nc.scalar.copy(ht[0:64, :], tp[0:64, :])
                nc.sync.dma_start(out[r0 : r0 + L, :], ht[0:64, :])
            yield

    # ---- weave: stage1(blk) emission is interleaved with stage234(blk-1)
    # so the PE's in-order queue alternates big main-matmul bursts with the
    # tiny latency-bound chain matmuls (keeps the PE dense and HAM-warm).
    avs = {}
    DONE = object()

    def adv(g, n=1):
        for _ in range(n):
            if next(g, DONE) is DONE:
                return False
        return True

    prev = None
    for blk in range(NBLK):
        cur = stage1_gen(blk)
        adv(cur)                      # embT DMA + a_sb alloc
        alive = prev is not None
        debt = 0
        while True:
            if not adv(cur):
                break
            if debt > 0:
                debt -= 1
                continue
            if alive:
                tok = next(prev, DONE)
                if tok is DONE:
                    alive = False
                elif tok == "slow":
                    debt = 2      # give slow units ~3 pieces of PE cover
        while alive:
            alive = adv(prev)
        prev = stage234_gen(blk)
    while adv(prev):
        pass


def kernel(emb, W, b, unnorm_trans, state_priors):
    emb = np.asarray(emb, dtype=np.float32)
    W = np.asarray(W, dtype=np.float32)
    b = np.asarray(b, dtype=np.float32)
    unnorm_trans = np.asarray(unnorm_trans, dtype=np.float32)
    state_priors = np.asarray(state_priors, dtype=np.float32)

    # host-side constants
    ut = unnorm_trans - unnorm_trans.max(axis=-1, keepdims=True)
    e = np.exp(ut)
    trans_half = (0.5 * e / e.sum(axis=-1, keepdims=True)).astype(np.float32)
    trans_rep = np.tile(trans_half, (2, 1)).astype(ml_dtypes.bfloat16)
    ident = np.tile(np.eye(H, dtype=np.float32), (2, 1)).astype(ml_dtypes.bfloat16)
    identf = np.eye(H, dtype=np.float32)
    onesr = np.ones((1, 128), dtype=np.float32)
    pr = np.exp(state_priors).astype(np.float32)
    priors_col = np.tile(pr[:, None], (2, 1)).astype(np.float32)  # [128, 1]
    priors_row = pr[None, :].astype(np.float32)

    # permute W columns so each m-tile q covers cols {(i, q), (i, q+32)} contiguously
    Wp = (
        W.reshape(D, H, H)
        .transpose(0, 2, 1)          # [d, j, i]
        .reshape(D, 2, 32, H)        # j = jsel*32 + q
        .transpose(0, 2, 1, 3)       # [d, q, jsel, i]
        .reshape(D, H * H)
    )
    Wp = np.ascontiguousarray(Wp).astype(ml_dtypes.bfloat16)
    # bias columns: bp[h*64 + i, 2q + h] = b[i*64 + q + 32h]  (tiled)
    bp64 = np.empty((H, 2 * 32), dtype=np.float32)
    br = b.reshape(H, H)
    for q in range(32):
        for h in range(2):
            bp64[:, 2 * q + h] = br[:, q + 32 * h]
    bp = np.ascontiguousarray(np.tile(bp64, (2, 1)))

    nc = build_bass()

    in_maps = []
    for core in range(NCORES):
        emb_c = emb[core * BLOC : (core + 1) * BLOC]          # [2, T, D]
        embT_c = np.ascontiguousarray(
            emb_c.transpose(2, 0, 1).reshape(D, BLOC * T)
        ).astype(ml_dtypes.bfloat16)
        in_maps.append(
            {
                "embT": embT_c,
                "Wp": Wp,
                "bp": bp,
                "trans_rep": trans_rep,
                "ident": ident,
                "identf": identf,
                "onesr": onesr,
                "priors_col": priors_col,
                "priors_row": priors_row,
            }
        )

    import os

    trace = bool(int(os.environ.get("KERNEL_TRACE", "0")))
    res = run_bass_kernel_spmd(nc, in_maps, list(range(NCORES)), trace=trace)
    global LAST_RESULTS
    LAST_RESULTS = res
    if trace and res.exec_time_ns is not None:
        print(f"HW exec time: {res.exec_time_ns} ns")
        print(f"  mean across cores: {res.mean_exec_time_ns} ns")
    outs = [res.results[i]["out"].reshape(BLOC, T, H) for i in range(NCORES)]
    h = np.concatenate(outs, axis=0).astype(np.float32)
    h[:, 0, :] = state_priors[None, :]
    return h
